# revision 37
# baseline (speedup 1.0000x reference)
"""Trainium2 Bass kernel for the masked-attention module.

Computation (per batch element, data-parallel over 8 NeuronCores):
    q  = query @ Wq.T + bq
    kp = key   @ Wk.T + bk
    v  = kp    @ Wv.T + bv          (quirk: values projected from projected key)
    s  = q @ kp.T / sqrt(D) + maskbias      (maskbias = -1e9 where mask==0)
    p  = softmax(s)                          (no max-subtraction needed: |s| is small,
                                              masked entries exp to exactly 0 in f32)
    out = p @ v
Returns (out, p) both f32.

Layout strategy per core:
  - query/key/weights are loaded TRANSPOSED straight from DRAM with
    strided descriptors (partition stride 4B -> 512B contiguous bursts),
    then cast f32->bf16 on VectorE into streaming operand tiles, so every
    matmul has its contraction dim on SBUF partitions.  Only the
    projected tensors (q^T, kp^T, v) stay resident in SBUF.
  - q/kp projections produce transposed (h-on-partitions) outputs by
    using the weight as the stationary operand.
  - score tiles [128 lq x 2048 lk] accumulate in PSUM over 8 h-chunks.
  - ScalarE computes exp (scale=1/sqrt(D) folded in) straight from PSUM;
    VectorE applies the 0/1 mask and produces per-row sums in one fused
    scalar_tensor_tensor pass (masked entries contribute exactly 0).
  - masked-exp tiles are transposed for the p @ v matmul with one batched
    xbar DMA transpose per lq tile; the 1/sum normalization (and the
    softmax row-sum==1 identity for the bv bias) is folded into the
    PSUM->SBUF copies of score and out.
"""

import math
from contextlib import ExitStack

import numpy as np

import concourse.bass as bass
import concourse.tile as tile
from concourse import bacc, mybir
from concourse.bass_utils import run_bass_kernel_spmd

B, LQ, LK, D = 8, 2048, 2048, 1024
P = 128
NLQ, NLK, ND, NH = LQ // P, LK // P, D // P, D // P
N_CORES = 8
FP = mybir.dt.float32
BF = mybir.dt.bfloat16
I32 = mybir.dt.int32
SCALE = 1.0 / math.sqrt(D)
CHUNK = 512  # PSUM bank = 512 f32


def build():
    nc = bacc.Bacc(
        "TRN2",
        target_bir_lowering=False,
        debug=False,
        enable_asserts=False,
        num_devices=N_CORES,
    )
    query = nc.dram_tensor("query", [LQ, D], FP, kind="ExternalInput")
    key = nc.dram_tensor("key", [LK, D], FP, kind="ExternalInput")
    mask = nc.dram_tensor("mask", [1, LK], I32, kind="ExternalInput")
    Wq = nc.dram_tensor("Wq", [D, D], FP, kind="ExternalInput")
    Wk = nc.dram_tensor("Wk", [D, D], FP, kind="ExternalInput")
    Wv = nc.dram_tensor("Wv", [D, D], FP, kind="ExternalInput")
    bq = nc.dram_tensor("bq", [1, D], FP, kind="ExternalInput")
    bk = nc.dram_tensor("bk", [1, D], FP, kind="ExternalInput")
    bv = nc.dram_tensor("bv", [1, D], FP, kind="ExternalInput")
    out_e = nc.dram_tensor("out", [LQ, D], FP, kind="ExternalOutput")
    score_e = nc.dram_tensor("score", [LQ, LK], FP, kind="ExternalOutput")

    with tile.TileContext(nc) as tc, ExitStack() as ctx:
        singles = ctx.enter_context(tc.tile_pool(name="singles", bufs=1))
        proj = ctx.enter_context(tc.tile_pool(name="proj", bufs=1))
        psum = ctx.enter_context(tc.tile_pool(name="psum", bufs=8, space="PSUM"))

        # persistent projected tensors
        kpT = [proj.tile([P, LK], BF, tag=f"kpT{i}", name=f"kpT{i}") for i in range(NH)]
        qT = [proj.tile([P, LQ], BF, tag=f"qT{i}", name=f"qT{i}") for i in range(NH)]
        v_sb = [proj.tile([P, D], BF, tag=f"v{t}", name=f"v{t}") for t in range(NLK)]

        # --- small constants ----------------------------------------------
        mask01b = singles.tile([P, LK], BF, tag="mask01b")
        bq_pb = singles.tile([P, NH], FP, tag="bq_pb")
        nc.gpsimd.dma_start(out=bq_pb, in_=bq[0, :].rearrange("(i p) -> p i", p=P))
        bk_pb = singles.tile([P, NH], FP, tag="bk_pb")
        nc.gpsimd.dma_start(out=bk_pb, in_=bk[0, :].rearrange("(i p) -> p i", p=P))
        bv_bc = singles.tile([P, D], FP, tag="bv_bc")

        with tc.tile_pool(name="maskprep", bufs=1) as maskprep:
            # mask broadcast across partitions as bf16 0/1 [128, LK]
            mask_ib = maskprep.tile([P, LK], I32, tag="mask_ib")
            mask_bcast_ap = bass.AP(
                tensor=mask.ap().tensor, offset=0, ap=[[0, P], [1, LK]]
            )
            nc.gpsimd.dma_start(out=mask_ib, in_=mask_bcast_ap)
            nc.vector.tensor_copy(out=mask01b, in_=mask_ib)
            bv_bcast_ap = bass.AP(
                tensor=bv.ap().tensor, offset=0, ap=[[0, P], [1, D]]
            )
            nc.gpsimd.dma_start(out=bv_bc, in_=bv_bcast_ap)

        with tc.tile_pool(name="prep", bufs=4) as prep, tc.tile_pool(
            name="wpool", bufs=1
        ) as wpool, tc.tile_pool(name="xstream", bufs=2) as xstream:
            # load f32 row-tile (SWDGE) -> cast bf16 (DVE) -> xbar transpose
            # SBUF->SBUF, alternating between the two HWDGE queues (SP/ACT)
            

            _ldq = [nc.gpsimd, nc.sync]
            _ldi = [0]

            def load_cast_pair(src, rt0):
                # one 1MB DMA (fans out over the SDMA engines) + one cast,
                # alternating issue between the SWDGE and SP-HWDGE paths
                f32t = prep.tile([P, 2, D], FP, tag="prep_f32", name="prep_f32", bufs=4)
                eng = _ldq[_ldi[0] % 2]
                _ldi[0] += 1
                eng.dma_start(
                    out=f32t,
                    in_=src[rt0 * P : (rt0 + 2) * P, :].rearrange(
                        "(r p) d -> p r d", p=P
                    ),
                )
                bft = prep.tile([P, 2, D], BF, tag="prep_bf", name="prep_bf", bufs=4)
                nc.vector.tensor_copy(out=bft, in_=f32t)
                return bft

            def stage_w(wt, w_src, g):
                # wt[do, rt, j, hh] = W[rt*128+hh, j*128+do]: wt[:, i, j, :]
                # is the [K=d-tile j, M=h-tile i] stationary operand.
                # Weight loads ride the third (ACT HWDGE) DMA path so they
                # never compete with the input streams.
                f32t = prep.tile([P, 2, D], FP, tag="prep_f32", name="prep_f32", bufs=4)
                nc.scalar.dma_start(
                    out=f32t,
                    in_=w_src[g * 2 * P : (g * 2 + 2) * P, :].rearrange(
                        "(r p) d -> p r d", p=P
                    ),
                )
                bft = prep.tile([P, 2, D], BF, tag="prep_bf", name="prep_bf", bufs=4)
                nc.vector.tensor_copy(out=bft, in_=f32t)
                for rr in range(2):
                    nc.sync.dma_start(
                        out=wt[:, g * 2 + rr, :, :], in_=bft[:, rr, :], transpose=True
                    )

            RPC = CHUNK // P  # row tiles per chunk

            # --- q/kp projections (streaming transposed activations),
            # software-pipelined: chunk m+1 staging and background weight
            # staging are emitted before chunk m's PSUM copies so the
            # in-order DVE/SP queues never stall the PE stream -----------
            def project(outT, x_src, wT, bias_pb, length, bg, pre=()):
                nmc = length // CHUNK
                xts = {}

                def stage(m):
                    # xt[do, r, j, row] = x[m*512 + r*128 + row, j*128 + do]
                    xt = xstream.tile([P, RPC, ND, P], BF, tag="xT", name="xT")
                    for g in range(RPC // 2):
                        bft = load_cast_pair(x_src, m * RPC + g * 2)
                        for rr in range(2):
                            nc.sync.dma_start(
                                out=xt[:, g * 2 + rr, :, :],
                                in_=bft[:, rr, :],
                                transpose=True,
                            )
                    xts[m] = xt

                stage(0)
                for item in pre:
                    stage_w(*item)
                for m in range(nmc):
                    if m + 1 < nmc:
                        stage(m + 1)
                    n_bg = -(-len(bg) // (nmc - m))  # finish bg by last chunk
                    for _ in range(n_bg):
                        stage_w(*bg.pop(0))
                    xt = xts.pop(m)
                    # two half-groups of 4 PSUM banks each, copies on the
                    # otherwise-idle ScalarE: bank recycling never blocks
                    # behind the DVE cast queue
                    for half in range(2):
                        i0 = half * (NH // 2)
                        pss = [
                            psum.tile([P, CHUNK], FP, tag="ps", name="ps")
                            for _ in range(NH // 2)
                        ]
                        for j in range(ND):
                            for ii in range(NH // 2):
                                nc.tensor.matmul(
                                    pss[ii],
                                    wT[:, i0 + ii, j, :],
                                    xt[:, :, j, :],
                                    start=(j == 0),
                                    stop=(j == ND - 1),
                                )
                        for ii in range(NH // 2):
                            i = i0 + ii
                            nc.scalar.activation(
                                out=outT[i][:, m * CHUNK : (m + 1) * CHUNK],
                                in_=pss[ii],
                                func=mybir.ActivationFunctionType.Identity,
                                bias=bias_pb[:, i : i + 1],
                            )

            # Order: kp -> v -> q.  v is pure PE work on resident tensors,
            # so q's input staging streams ahead during the whole v phase.
            # Ring slots (tag wAB, bufs=2): wkT(A), wvT(B), wqT(A) — wqT's
            # WAR wait is on the kp matmuls, done long before q starts;
            # wvT has a fresh slot so it stages during kp.
            wkT = wpool.tile([P, NH, ND, P], BF, tag="wAB", bufs=2, name="wkT")
            wvT = wpool.tile([P, NH, ND, P], BF, tag="wAB", bufs=2, name="wvT")
            wqT = wpool.tile([P, NH, ND, P], BF, tag="wAB", bufs=2, name="wqT")
            bg = [(wkT, Wk, 2), (wkT, Wk, 3)] + [(wvT, Wv, g) for g in range(4)]
            project(
                kpT, key, wkT, bk_pb, LK, bg, pre=[(wkT, Wk, 0), (wkT, Wk, 1)]
            )
            assert not bg

            # --- v[lk, o] = kp @ Wv.T + bv (natural layout); wqT and q's
            # first chunks stage in the shadow of these matmuls ---------------
            for t in range(NLK):
                if t % 4 == 0:
                    stage_w(wqT, Wq, t // 4)
                nch = D // CHUNK
                pss = [
                    psum.tile([P, CHUNK], FP, tag="ps", name="ps") for _ in range(nch)
                ]
                for j in range(NH):
                    lhsT = kpT[j][:, t * P : (t + 1) * P]
                    for c in range(nch):
                        nc.tensor.matmul(
                            pss[c],
                            lhsT,
                            wvT[:, 4 * c : 4 * c + 4, j, :],
                            start=(j == 0),
                            stop=(j == NH - 1),
                        )
                for c in range(nch):
                    nc.scalar.copy(
                        out=v_sb[t][:, c * CHUNK : (c + 1) * CHUNK], in_=pss[c]
                    )

            project(qT, query, wqT, bq_pb, LQ, [])

        # --- attention: score / softmax / output ---------------------------
        sb = ctx.enter_context(tc.tile_pool(name="sb", bufs=2))
        sb3 = ctx.enter_context(tc.tile_pool(name="sb3", bufs=3))
        small = ctx.enter_context(tc.tile_pool(name="small", bufs=4))

        state = {}

        def emit_score(t):
            nch = LK // CHUNK
            pss = [psum.tile([P, CHUNK], FP, tag="ps", name="ps") for _ in range(nch)]
            for j in range(NH):
                lhsT = qT[j][:, t * P : (t + 1) * P]
                for c in range(nch):
                    nc.tensor.matmul(
                        pss[c],
                        lhsT,
                        kpT[j][:, c * CHUNK : (c + 1) * CHUNK],
                        start=(j == 0),
                        stop=(j == NH - 1),
                    )
            exp_t = sb.tile([P, LK], BF, tag="exp")
            for c in range(nch):
                nc.scalar.activation(
                    out=exp_t[:, c * CHUNK : (c + 1) * CHUNK],
                    in_=pss[c],
                    func=mybir.ActivationFunctionType.Exp,
                    scale=SCALE,
                )
            # masked exp + per-chunk row sums in one DVE pass
            expm = sb.tile([P, LK], BF, tag="expm")
            sums4 = small.tile([P, nch], FP, tag="sums4")
            for c in range(nch):
                nc.vector.scalar_tensor_tensor(
                    out=expm[:, c * CHUNK : (c + 1) * CHUNK],
                    in0=exp_t[:, c * CHUNK : (c + 1) * CHUNK],
                    scalar=1.0,
                    in1=mask01b[:, c * CHUNK : (c + 1) * CHUNK],
                    op0=mybir.AluOpType.mult,
                    op1=mybir.AluOpType.mult,
                    accum_out=sums4[:, c : c + 1],
                )
            sums = small.tile([P, 1], FP, tag="sums")
            nc.vector.reduce_sum(out=sums, in_=sums4, axis=mybir.AxisListType.X)
            recip = small.tile([P, 1], FP, tag="recip")
            nc.vector.reciprocal(out=recip, in_=sums)
            expT = sb3.tile([P, NLK, P], BF, tag="expT")
            nc.sync.dma_start(out=expT[:], in_=expm[:], transpose=True)
            score_f = sb.tile([P, LK], FP, tag="score_f")
            nc.vector.tensor_scalar_mul(out=score_f, in0=expm, scalar1=recip)
            nc.gpsimd.dma_start(out=score_e[t * P : (t + 1) * P, :], in_=score_f)
            state[t] = (expT, recip)

        def emit_out(t):
            expT, recip = state.pop(t)
            nch = D // CHUNK
            pso = [psum.tile([P, CHUNK], FP, tag="ps", name="ps") for _ in range(nch)]
            for b in range(NLK):
                lhsT = expT[:, b, :]
                for c in range(nch):
                    nc.tensor.matmul(
                        pso[c],
                        lhsT,
                        v_sb[b][:, c * CHUNK : (c + 1) * CHUNK],
                        start=(b == 0),
                        stop=(b == NLK - 1),
                    )
            out_f = sb.tile([P, D], FP, tag="out_f")
            for c in range(nch):
                nc.vector.scalar_tensor_tensor(
                    out=out_f[:, c * CHUNK : (c + 1) * CHUNK],
                    in0=pso[c],
                    scalar=recip,
                    in1=bv_bc[:, c * CHUNK : (c + 1) * CHUNK],
                    op0=mybir.AluOpType.mult,
                    op1=mybir.AluOpType.add,
                )
            nc.gpsimd.dma_start(out=out_e[t * P : (t + 1) * P, :], in_=out_f)

        for t in range(NLQ):
            emit_score(t)
            if t >= 1:
                emit_out(t - 1)
        emit_out(NLQ - 1)

    nc.compile()
    return nc


_NC = None


def _get_nc():
    global _NC
    if _NC is None:
        _NC = build()
    return _NC


def make_in_maps(key, query, mask, Wq, bq, Wk, bk, Wv, bv):
    key = np.asarray(key, dtype=np.float32)
    query = np.asarray(query, dtype=np.float32)
    mask = np.asarray(mask, dtype=np.int32)
    ws = {
        "Wq": np.ascontiguousarray(np.asarray(Wq, dtype=np.float32)),
        "Wk": np.ascontiguousarray(np.asarray(Wk, dtype=np.float32)),
        "Wv": np.ascontiguousarray(np.asarray(Wv, dtype=np.float32)),
        "bq": np.asarray(bq, dtype=np.float32).reshape(1, D),
        "bk": np.asarray(bk, dtype=np.float32).reshape(1, D),
        "bv": np.asarray(bv, dtype=np.float32).reshape(1, D),
    }
    return [
        {
            "query": np.ascontiguousarray(query[i]),
            "key": np.ascontiguousarray(key[i]),
            "mask": np.ascontiguousarray(mask[i].reshape(1, LK)),
            **ws,
        }
        for i in range(N_CORES)
    ]


def kernel(key, query, mask, Wq, bq, Wk, bk, Wv, bv):
    nc = _get_nc()
    in_maps = make_in_maps(key, query, mask, Wq, bq, Wk, bk, Wv, bv)
    res = run_bass_kernel_spmd(nc, in_maps, list(range(N_CORES))).results
    output = np.stack([np.asarray(res[i]["out"]) for i in range(N_CORES)])
    score = np.stack([np.asarray(res[i]["score"]) for i in range(N_CORES)])
    return output, score


# revision 38
# speedup vs baseline: 1.0780x; 1.0780x over previous
"""Trainium2 Bass kernel for the masked-attention module.

Computation (per batch element, data-parallel over 8 NeuronCores):
    q  = query @ Wq.T + bq
    kp = key   @ Wk.T + bk
    v  = kp    @ Wv.T + bv          (quirk: values projected from projected key)
    s  = q @ kp.T / sqrt(D) + maskbias      (maskbias = -1e9 where mask==0)
    p  = softmax(s)                          (no max-subtraction needed: |s| is small,
                                              masked entries exp to exactly 0 in f32)
    out = p @ v
Returns (out, p) both f32.

Layout strategy per core:
  - query/key/weights are loaded TRANSPOSED straight from DRAM with
    strided descriptors (partition stride 4B -> 512B contiguous bursts),
    then cast f32->bf16 on VectorE into streaming operand tiles, so every
    matmul has its contraction dim on SBUF partitions.  Only the
    projected tensors (q^T, kp^T, v) stay resident in SBUF.
  - q/kp projections produce transposed (h-on-partitions) outputs by
    using the weight as the stationary operand.
  - score tiles [128 lq x 2048 lk] accumulate in PSUM over 8 h-chunks.
  - ScalarE computes exp (scale=1/sqrt(D) folded in) straight from PSUM;
    VectorE applies the 0/1 mask and produces per-row sums in one fused
    scalar_tensor_tensor pass (masked entries contribute exactly 0).
  - masked-exp tiles are transposed for the p @ v matmul with one batched
    xbar DMA transpose per lq tile; the 1/sum normalization (and the
    softmax row-sum==1 identity for the bv bias) is folded into the
    PSUM->SBUF copies of score and out.
"""

import math
from contextlib import ExitStack

import numpy as np

import concourse.bass as bass
import concourse.tile as tile
from concourse import bacc, mybir
from concourse.bass_utils import run_bass_kernel_spmd

B, LQ, LK, D = 8, 2048, 2048, 1024
P = 128
NLQ, NLK, ND, NH = LQ // P, LK // P, D // P, D // P
N_CORES = 8
FP = mybir.dt.float32
BF = mybir.dt.bfloat16
I32 = mybir.dt.int32
SCALE = 1.0 / math.sqrt(D)
CHUNK = 512  # PSUM bank = 512 f32


def build():
    nc = bacc.Bacc(
        "TRN2",
        target_bir_lowering=False,
        debug=False,
        enable_asserts=False,
        num_devices=N_CORES,
    )
    query = nc.dram_tensor("query", [LQ, D], FP, kind="ExternalInput")
    key = nc.dram_tensor("key", [LK, D], FP, kind="ExternalInput")
    mask = nc.dram_tensor("mask", [1, LK], I32, kind="ExternalInput")
    Wq = nc.dram_tensor("Wq", [D, D], FP, kind="ExternalInput")
    Wk = nc.dram_tensor("Wk", [D, D], FP, kind="ExternalInput")
    Wv = nc.dram_tensor("Wv", [D, D], FP, kind="ExternalInput")
    bq = nc.dram_tensor("bq", [1, D], FP, kind="ExternalInput")
    bk = nc.dram_tensor("bk", [1, D], FP, kind="ExternalInput")
    bv = nc.dram_tensor("bv", [1, D], FP, kind="ExternalInput")
    out_e = nc.dram_tensor("out", [LQ, D], FP, kind="ExternalOutput")
    score_e = nc.dram_tensor("score", [LQ, LK], FP, kind="ExternalOutput")

    with tile.TileContext(nc) as tc, ExitStack() as ctx:
        singles = ctx.enter_context(tc.tile_pool(name="singles", bufs=1))
        proj = ctx.enter_context(tc.tile_pool(name="proj", bufs=1))
        psum = ctx.enter_context(tc.tile_pool(name="psum", bufs=8, space="PSUM"))

        # persistent projected tensors
        kpT = [proj.tile([P, LK], BF, tag=f"kpT{i}", name=f"kpT{i}") for i in range(NH)]
        qT = [proj.tile([P, LQ], BF, tag=f"qT{i}", name=f"qT{i}") for i in range(NH)]
        v_sb = [proj.tile([P, D], BF, tag=f"v{t}", name=f"v{t}") for t in range(NLK)]

        # --- small constants ----------------------------------------------
        mask01b = singles.tile([P, LK], BF, tag="mask01b")
        bq_pb = singles.tile([P, NH], FP, tag="bq_pb")
        nc.gpsimd.dma_start(out=bq_pb, in_=bq[0, :].rearrange("(i p) -> p i", p=P))
        bk_pb = singles.tile([P, NH], FP, tag="bk_pb")
        nc.gpsimd.dma_start(out=bk_pb, in_=bk[0, :].rearrange("(i p) -> p i", p=P))
        bv_bc = singles.tile([P, D], FP, tag="bv_bc")

        with tc.tile_pool(name="maskprep", bufs=1) as maskprep:
            # mask broadcast across partitions as bf16 0/1 [128, LK]
            mask_ib = maskprep.tile([P, LK], I32, tag="mask_ib")
            mask_bcast_ap = bass.AP(
                tensor=mask.ap().tensor, offset=0, ap=[[0, P], [1, LK]]
            )
            nc.gpsimd.dma_start(out=mask_ib, in_=mask_bcast_ap)
            nc.vector.tensor_copy(out=mask01b, in_=mask_ib)
            bv_bcast_ap = bass.AP(
                tensor=bv.ap().tensor, offset=0, ap=[[0, P], [1, D]]
            )
            nc.gpsimd.dma_start(out=bv_bc, in_=bv_bcast_ap)

        with tc.tile_pool(name="prep", bufs=4) as prep, tc.tile_pool(
            name="wpool", bufs=1
        ) as wpool, tc.tile_pool(name="xstream", bufs=2) as xstream:
            # load f32 row-tile (SWDGE) -> cast bf16 (DVE) -> xbar transpose
            # SBUF->SBUF, alternating between the two HWDGE queues (SP/ACT)
            

            _ldq = [nc.gpsimd, nc.sync]
            _ldi = [0]

            def load_cast_pair(src, rt0):
                # one 1MB DMA (fans out over the SDMA engines) + one cast,
                # alternating issue between the SWDGE and SP-HWDGE paths
                f32t = prep.tile([P, 2, D], FP, tag="prep_f32", name="prep_f32", bufs=4)
                eng = _ldq[_ldi[0] % 2]
                _ldi[0] += 1
                eng.dma_start(
                    out=f32t,
                    in_=src[rt0 * P : (rt0 + 2) * P, :].rearrange(
                        "(r p) d -> p r d", p=P
                    ),
                )
                bft = prep.tile([P, 2, D], BF, tag="prep_bf", name="prep_bf", bufs=4)
                nc.vector.tensor_copy(out=bft, in_=f32t)
                return bft

            def stage_w(wt, w_src, g):
                # wt[do, rt, j, hh] = W[rt*128+hh, j*128+do]: wt[:, i, j, :]
                # is the [K=d-tile j, M=h-tile i] stationary operand
                bft = load_cast_pair(w_src, g * 2)
                for rr in range(2):
                    nc.sync.dma_start(
                        out=wt[:, g * 2 + rr, :, :], in_=bft[:, rr, :], transpose=True
                    )

            RPC = CHUNK // P  # row tiles per chunk

            # --- q/kp projections (streaming transposed activations),
            # software-pipelined: chunk m+1 staging and background weight
            # staging are emitted before chunk m's PSUM copies so the
            # in-order DVE/SP queues never stall the PE stream -----------
            def project(outT, x_src, wT, bias_pb, length, bg, pre=()):
                nmc = length // CHUNK
                xts = {}

                def stage(m):
                    # xt[do, r, j, row] = x[m*512 + r*128 + row, j*128 + do]
                    xt = xstream.tile([P, RPC, ND, P], BF, tag="xT", name="xT")
                    for g in range(RPC // 2):
                        bft = load_cast_pair(x_src, m * RPC + g * 2)
                        for rr in range(2):
                            nc.sync.dma_start(
                                out=xt[:, g * 2 + rr, :, :],
                                in_=bft[:, rr, :],
                                transpose=True,
                            )
                    xts[m] = xt

                stage(0)
                for item in pre:
                    stage_w(*item)
                for m in range(nmc):
                    if m + 1 < nmc:
                        stage(m + 1)
                    n_bg = -(-len(bg) // (nmc - m))  # finish bg by last chunk
                    for _ in range(n_bg):
                        stage_w(*bg.pop(0))
                    xt = xts.pop(m)
                    # two half-groups of 4 PSUM banks each, copies on the
                    # otherwise-idle ScalarE: bank recycling never blocks
                    # behind the DVE cast queue
                    for half in range(2):
                        i0 = half * (NH // 2)
                        pss = [
                            psum.tile([P, CHUNK], FP, tag="ps", name="ps")
                            for _ in range(NH // 2)
                        ]
                        for j in range(ND):
                            for ii in range(NH // 2):
                                nc.tensor.matmul(
                                    pss[ii],
                                    wT[:, i0 + ii, j, :],
                                    xt[:, :, j, :],
                                    start=(j == 0),
                                    stop=(j == ND - 1),
                                )
                        for ii in range(NH // 2):
                            i = i0 + ii
                            nc.scalar.activation(
                                out=outT[i][:, m * CHUNK : (m + 1) * CHUNK],
                                in_=pss[ii],
                                func=mybir.ActivationFunctionType.Identity,
                                bias=bias_pb[:, i : i + 1],
                            )

            # Order: kp -> v -> q.  v is pure PE work on resident tensors,
            # so q's input staging streams ahead during the whole v phase.
            # Ring slots (tag wAB, bufs=2): wkT(A), wvT(B), wqT(A) — wqT's
            # WAR wait is on the kp matmuls, done long before q starts;
            # wvT has a fresh slot so it stages during kp.
            wkT = wpool.tile([P, NH, ND, P], BF, tag="wAB", bufs=2, name="wkT")
            wvT = wpool.tile([P, NH, ND, P], BF, tag="wAB", bufs=2, name="wvT")
            wqT = wpool.tile([P, NH, ND, P], BF, tag="wAB", bufs=2, name="wqT")
            bg = [(wkT, Wk, 2), (wkT, Wk, 3)] + [(wvT, Wv, g) for g in range(4)]
            project(
                kpT, key, wkT, bk_pb, LK, bg, pre=[(wkT, Wk, 0), (wkT, Wk, 1)]
            )
            assert not bg

            # --- v[lk, o] = kp @ Wv.T + bv (natural layout); wqT and q's
            # first chunks stage in the shadow of these matmuls ---------------
            for t in range(NLK):
                if t % 4 == 0:
                    stage_w(wqT, Wq, t // 4)
                nch = D // CHUNK
                pss = [
                    psum.tile([P, CHUNK], FP, tag="ps", name="ps") for _ in range(nch)
                ]
                for j in range(NH):
                    lhsT = kpT[j][:, t * P : (t + 1) * P]
                    for c in range(nch):
                        nc.tensor.matmul(
                            pss[c],
                            lhsT,
                            wvT[:, 4 * c : 4 * c + 4, j, :],
                            start=(j == 0),
                            stop=(j == NH - 1),
                        )
                for c in range(nch):
                    nc.scalar.copy(
                        out=v_sb[t][:, c * CHUNK : (c + 1) * CHUNK], in_=pss[c]
                    )

            project(qT, query, wqT, bq_pb, LQ, [])

        # --- attention: score / softmax / output ---------------------------
        sb = ctx.enter_context(tc.tile_pool(name="sb", bufs=2))
        sb3 = ctx.enter_context(tc.tile_pool(name="sb3", bufs=3))
        small = ctx.enter_context(tc.tile_pool(name="small", bufs=4))

        state = {}

        def emit_score(t):
            nch = LK // CHUNK
            pss = [psum.tile([P, CHUNK], FP, tag="ps", name="ps") for _ in range(nch)]
            for j in range(NH):
                lhsT = qT[j][:, t * P : (t + 1) * P]
                for c in range(nch):
                    nc.tensor.matmul(
                        pss[c],
                        lhsT,
                        kpT[j][:, c * CHUNK : (c + 1) * CHUNK],
                        start=(j == 0),
                        stop=(j == NH - 1),
                    )
            exp_t = sb.tile([P, LK], BF, tag="exp")
            for c in range(nch):
                nc.scalar.activation(
                    out=exp_t[:, c * CHUNK : (c + 1) * CHUNK],
                    in_=pss[c],
                    func=mybir.ActivationFunctionType.Exp,
                    scale=SCALE,
                )
            # masked exp + per-chunk row sums in one DVE pass
            expm = sb.tile([P, LK], BF, tag="expm")
            sums4 = small.tile([P, nch], FP, tag="sums4")
            for c in range(nch):
                nc.vector.scalar_tensor_tensor(
                    out=expm[:, c * CHUNK : (c + 1) * CHUNK],
                    in0=exp_t[:, c * CHUNK : (c + 1) * CHUNK],
                    scalar=1.0,
                    in1=mask01b[:, c * CHUNK : (c + 1) * CHUNK],
                    op0=mybir.AluOpType.mult,
                    op1=mybir.AluOpType.mult,
                    accum_out=sums4[:, c : c + 1],
                )
            sums = small.tile([P, 1], FP, tag="sums")
            nc.vector.reduce_sum(out=sums, in_=sums4, axis=mybir.AxisListType.X)
            recip = small.tile([P, 1], FP, tag="recip")
            nc.vector.reciprocal(out=recip, in_=sums)
            expT = sb3.tile([P, NLK, P], BF, tag="expT")
            nc.sync.dma_start(out=expT[:], in_=expm[:], transpose=True)
            score_f = sb.tile([P, LK], FP, tag="score_f")
            nc.vector.tensor_scalar_mul(out=score_f, in0=expm, scalar1=recip)
            nc.gpsimd.dma_start(out=score_e[t * P : (t + 1) * P, :], in_=score_f)
            state[t] = (expT, recip)

        def emit_out(t):
            expT, recip = state.pop(t)
            nch = D // CHUNK
            pso = [psum.tile([P, CHUNK], FP, tag="ps", name="ps") for _ in range(nch)]
            for b in range(NLK):
                lhsT = expT[:, b, :]
                for c in range(nch):
                    nc.tensor.matmul(
                        pso[c],
                        lhsT,
                        v_sb[b][:, c * CHUNK : (c + 1) * CHUNK],
                        start=(b == 0),
                        stop=(b == NLK - 1),
                    )
            out_f = sb.tile([P, D], FP, tag="out_f")
            for c in range(nch):
                nc.vector.scalar_tensor_tensor(
                    out=out_f[:, c * CHUNK : (c + 1) * CHUNK],
                    in0=pso[c],
                    scalar=recip,
                    in1=bv_bc[:, c * CHUNK : (c + 1) * CHUNK],
                    op0=mybir.AluOpType.mult,
                    op1=mybir.AluOpType.add,
                )
            nc.gpsimd.dma_start(out=out_e[t * P : (t + 1) * P, :], in_=out_f)

        for t in range(NLQ):
            emit_score(t)
            if t >= 1:
                emit_out(t - 1)
        emit_out(NLQ - 1)

    nc.compile()
    return nc


_NC = None


def _get_nc():
    global _NC
    if _NC is None:
        _NC = build()
    return _NC


def make_in_maps(key, query, mask, Wq, bq, Wk, bk, Wv, bv):
    key = np.asarray(key, dtype=np.float32)
    query = np.asarray(query, dtype=np.float32)
    mask = np.asarray(mask, dtype=np.int32)
    ws = {
        "Wq": np.ascontiguousarray(np.asarray(Wq, dtype=np.float32)),
        "Wk": np.ascontiguousarray(np.asarray(Wk, dtype=np.float32)),
        "Wv": np.ascontiguousarray(np.asarray(Wv, dtype=np.float32)),
        "bq": np.asarray(bq, dtype=np.float32).reshape(1, D),
        "bk": np.asarray(bk, dtype=np.float32).reshape(1, D),
        "bv": np.asarray(bv, dtype=np.float32).reshape(1, D),
    }
    return [
        {
            "query": np.ascontiguousarray(query[i]),
            "key": np.ascontiguousarray(key[i]),
            "mask": np.ascontiguousarray(mask[i].reshape(1, LK)),
            **ws,
        }
        for i in range(N_CORES)
    ]


def kernel(key, query, mask, Wq, bq, Wk, bk, Wv, bv):
    nc = _get_nc()
    in_maps = make_in_maps(key, query, mask, Wq, bq, Wk, bk, Wv, bv)
    res = run_bass_kernel_spmd(nc, in_maps, list(range(N_CORES))).results
    output = np.stack([np.asarray(res[i]["out"]) for i in range(N_CORES)])
    score = np.stack([np.asarray(res[i]["score"]) for i in range(N_CORES)])
    return output, score
